# revision 4
# baseline (speedup 1.0000x reference)
"""DCT block extractor kernel for 8 TRN2 NeuronCores (pure data parallel).

Math: for each 8x8 block of each [512,512] image, the 2D-DFT bin (u,v) is
  X[u,v] = sum_{r,s} x[r,s] * exp(-2*pi*i*(u*r + v*s)/8)
We need |X| at 6 (u,v) bands, averaged over all 64x64 blocks.

Implementation: contraction over the in-block row index r is done on the
TensorEngine partition axis (block-diagonal weights over 8 row-groups per
64-row chunk); contraction over the in-block column index s is done by PSUM
accumulation across 8 matmuls, each reading a stride-8 column slice of the
image rows. One matmul per (chunk, s):
  lhsT = W[s]  [64, 96]   (k = gi*8+r, m = part*8+gi; part = band / 6+band)
  rhs  = rows[:, s::8]    [64, 384]   (free = (img in batch, gj))
The 96 PSUM partitions hold Re (0:48) and Im (48:96) per (band, gi).
Magnitude via ScalarE Square/Sqrt, accumulate + gj-reduce on VectorE.
Final tiny mean/reshape is done on host from a [48, 24] per-core result.
"""

import os
import sys

import numpy as np

for _p in ("/opt/trn_rl_repo",):
    if os.path.isdir(_p) and _p not in sys.path:
        sys.path.insert(0, _p)

import concourse.bass as bass  # noqa: E402
import concourse.tile as tile  # noqa: E402
from concourse import bacc, mybir  # noqa: E402
from concourse.bass_utils import run_bass_kernel_spmd  # noqa: E402

# Problem shape (hardcoded per contract)
B, C, H, W = 64, 3, 512, 512
N_CORES = 8
BL = B // N_CORES  # 8 batch rows per core
BPB = 2            # batch elements per device-batch
IPB = BPB * C      # 6 images per device-batch
NBATCH = BL // BPB  # 4 device-batches
NCHUNK = 8         # 64-row chunks per image
GJ = 64            # block-columns
NFREE = IPB * GJ   # 384 matmul free size
NBANDS = 6
NIMG = BL * C      # 24 images per core

FREQ_BANDS = np.array([[0, 1], [1, 0], [1, 1], [2, 2], [3, 3], [4, 4]]) % 8

BENCH = False          # set True (e.g. from test.py) to profile
BENCH_KWARGS = {}
LAST_EXEC_NS = None
LAST_RESULTS = None

_CACHED_NC = None


def _weights() -> np.ndarray:
    """W[s] in [8, 128, 128]: Re at m=band*8+gi, Im at m=64+band*8+gi.

    Rows 64:128 duplicate rows 0:64 so lhsT can be sliced at base partition
    0 or 64 to match the rhs chunk's base partition."""
    w = np.zeros((8, 64, 128), dtype=np.float32)
    r = np.arange(8)
    for s in range(8):
        for b, (u, v) in enumerate(FREQ_BANDS):
            th = 2.0 * np.pi * (u * r + v * s) / 8.0
            cs, sn = np.cos(th), np.sin(th)
            for gi in range(8):
                w[s, gi * 8 : gi * 8 + 8, b * 8 + gi] = cs
                w[s, gi * 8 : gi * 8 + 8, 64 + b * 8 + gi] = sn
    return np.concatenate([w, w], axis=1)


def _build():
    nc = bacc.Bacc("TRN2", target_bir_lowering=False, debug=False, num_devices=N_CORES)
    f32 = mybir.dt.float32
    f32r = mybir.dt.float32r

    x_d = nc.dram_tensor("x", [BL, C, H, W], f32, kind="ExternalInput")
    w_d = nc.dram_tensor("w", [8, 128, 128], f32, kind="ExternalInput")
    out_d = nc.dram_tensor("out", [48, NIMG], f32, kind="ExternalOutput")

    with tile.TileContext(nc) as tc:
        with (
            tc.tile_pool(name="consts", bufs=1) as consts,
            tc.tile_pool(name="inp", bufs=2) as inp,
            tc.tile_pool(name="psum", bufs=8, space="PSUM") as psum_pool,
            tc.tile_pool(name="work", bufs=3) as work,
            tc.tile_pool(name="accp", bufs=2) as accp,
            tc.tile_pool(name="outp", bufs=2) as outp,
        ):
            w_sb = consts.tile([128, 8, 128], f32r)
            nc.sync.dma_start(out=w_sb, in_=w_d[:].transpose([1, 0, 2]).bitcast(f32r))

            for bt in range(NBATCH):
                tiles = []
                for t in range(4):  # each tile holds chunks 2t (p 0:64), 2t+1 (p 64:128)
                    it = inp.tile([128, BPB, C, W], f32r)
                    for half in range(2):
                        ch = 2 * t + half
                        nc.sync.dma_start(
                            out=it[64 * half : 64 * half + 64],
                            in_=x_d[
                                bt * BPB : (bt + 1) * BPB, :, 64 * ch : 64 * ch + 64, :
                            ].transpose([2, 0, 1, 3]).bitcast(f32r),
                        )
                    tiles.append(it)

                acc = accp.tile([48, NFREE], f32)
                for chunk in range(NCHUNK):
                    it = tiles[chunk // 2]
                    base = 64 * (chunk % 2)
                    # [64, (b c), gj, s] -> stride-8 column-phase slices
                    rhs_v = it[base : base + 64].rearrange(
                        "k b c (g s) -> k (b c) g s", s=8
                    )
                    ps = psum_pool.tile([128, NFREE], f32)
                    for s in range(8):
                        nc.tensor.matmul(
                            ps,
                            w_sb[base : base + 64, s, :],
                            rhs_v[:, :, :, s],
                            start=(s == 0),
                            stop=(s == 7),
                        )
                    sq_re = work.tile([48, NFREE], f32)
                    sq_im = work.tile([48, NFREE], f32)
                    nc.scalar.square(sq_re, ps[0:48])
                    nc.scalar.square(sq_im, ps[64:112])
                    ss = work.tile([48, NFREE], f32)
                    nc.vector.tensor_add(ss, sq_re, sq_im)
                    if chunk == 0:
                        nc.scalar.sqrt(acc, ss)
                    else:
                        mag = work.tile([48, NFREE], f32)
                        nc.scalar.sqrt(mag, ss)
                        nc.vector.tensor_add(acc, acc, mag)

                ob = outp.tile([48, IPB], f32)
                nc.vector.reduce_sum(
                    out=ob,
                    in_=acc.rearrange("p (i g) -> p i g", g=GJ),
                    axis=mybir.AxisListType.X,
                )
                nc.sync.dma_start(
                    out=out_d[:, bt * IPB : (bt + 1) * IPB], in_=ob
                )

    nc.compile()
    return nc


def kernel(x: np.ndarray) -> np.ndarray:
    global _CACHED_NC, LAST_EXEC_NS, LAST_RESULTS
    x = np.ascontiguousarray(np.asarray(x, dtype=np.float32))
    assert x.shape == (B, C, H, W), x.shape

    if _CACHED_NC is None:
        _CACHED_NC = _build()
    nc = _CACHED_NC

    w = _weights()
    in_maps = [
        {"x": x[i * BL : (i + 1) * BL], "w": w} for i in range(N_CORES)
    ]
    kwargs = dict(BENCH_KWARGS)
    if BENCH:
        kwargs.setdefault("trace", True)
    res = run_bass_kernel_spmd(nc, in_maps, core_ids=list(range(N_CORES)), **kwargs)
    LAST_EXEC_NS = res.exec_time_ns
    LAST_RESULTS = res

    outs = []
    for i in range(N_CORES):
        o = np.asarray(res.results[i]["out"], dtype=np.float64)  # [48, 24]
        o = o.reshape(NBANDS, 8, NBATCH, BPB, C)  # [band, gi_l, bt, b_idx, ch]
        o = o.sum(axis=1) / 4096.0                # mean over all 64x64 blocks
        o = np.transpose(o, (1, 2, 3, 0))         # [bt, b_idx, ch, band]
        outs.append(o.reshape(BL, C * NBANDS))
    return np.concatenate(outs, axis=0).astype(np.float32)


# revision 6
# speedup vs baseline: 1.3390x; 1.3390x over previous
"""DCT block extractor kernel for 8 TRN2 NeuronCores (pure data parallel).

Math: for each 8x8 block of each [512,512] image, the 2D-DFT bin (u,v) is
  X[u,v] = sum_{r,s} x[r,s] * exp(-2*pi*i*(u*r + v*s)/8)
We need |X| at 6 (u,v) bands, averaged over all 64x64 blocks.

Implementation: contraction over the in-block row index r is done on the
TensorEngine partition axis (block-diagonal weights over 8 row-groups per
64-row chunk); contraction over the in-block column index s is done by PSUM
accumulation across 8 matmuls, each reading a stride-8 column slice of the
image rows. One matmul per (chunk, s):
  lhsT = W[s]  [64, 128]  (k = gi*8+r; Re at m=band*8+gi, Im at m=64+band*8+gi)
  rhs  = rows[:, s::8]    [64, 512]   (free = (img in batch, gj))
Inputs are cast fp32->fp16 by the (gpsimd software-DGE) DMA so the matmul
runs single-pass at 1 cycle/row with fast weight load; PSUM accumulates fp32.
Magnitude via ScalarE Square/Sqrt, accumulate + gj-reduce on VectorE.
Final tiny mean/reshape is done on host from a [48, 24] per-core result.
"""

import os
import sys

import numpy as np

for _p in ("/opt/trn_rl_repo",):
    if os.path.isdir(_p) and _p not in sys.path:
        sys.path.insert(0, _p)

import concourse.bass as bass  # noqa: E402
import concourse.tile as tile  # noqa: E402
from concourse import bacc, mybir  # noqa: E402
from concourse.bass_utils import run_bass_kernel_spmd  # noqa: E402

# Problem shape (hardcoded per contract)
B, C, H, W = 64, 3, 512, 512
N_CORES = 8
BL = B // N_CORES   # 8 batch rows per core
NIMG = BL * C       # 24 images per core (flattened (b, c))
IPB = 8             # images per device-batch
NBATCH = NIMG // IPB  # 3 device-batches
NCHUNK = 8          # 64-row chunks per image
GJ = 64             # block-columns
NFREE = IPB * GJ    # 512 matmul free size
NBANDS = 6

FREQ_BANDS = np.array([[0, 1], [1, 0], [1, 1], [2, 2], [3, 3], [4, 4]]) % 8

BENCH = False          # set True (e.g. from test.py) to profile
BENCH_KWARGS = {}
LAST_EXEC_NS = None
LAST_RESULTS = None

_CACHED_NC = None


def _weights() -> np.ndarray:
    """W[s] in [8, 128, 128] fp16: Re at m=band*8+gi, Im at m=64+band*8+gi.

    Rows 64:128 duplicate rows 0:64 so lhsT can be sliced at base partition
    0 or 64 to match the rhs chunk's base partition."""
    w = np.zeros((8, 64, 128), dtype=np.float32)
    r = np.arange(8)
    for s in range(8):
        for b, (u, v) in enumerate(FREQ_BANDS):
            th = 2.0 * np.pi * (u * r + v * s) / 8.0
            cs, sn = np.cos(th), np.sin(th)
            for gi in range(8):
                w[s, gi * 8 : gi * 8 + 8, b * 8 + gi] = cs
                w[s, gi * 8 : gi * 8 + 8, 64 + b * 8 + gi] = sn
    return np.concatenate([w, w], axis=1).astype(np.float16)


def _build():
    nc = bacc.Bacc("TRN2", target_bir_lowering=False, debug=False, num_devices=N_CORES)
    f32 = mybir.dt.float32
    f16 = mybir.dt.float16

    x_d = nc.dram_tensor("x", [NIMG, H, W], f32, kind="ExternalInput")
    w_d = nc.dram_tensor("w", [8, 128, 128], f16, kind="ExternalInput")
    out_d = nc.dram_tensor("out", [48, NIMG], f32, kind="ExternalOutput")

    with tile.TileContext(nc) as tc:
        with (
            tc.tile_pool(name="consts", bufs=1) as consts,
            tc.tile_pool(name="inp", bufs=2) as inp,
            tc.tile_pool(name="psum", bufs=8, space="PSUM") as psum_pool,
            tc.tile_pool(name="work", bufs=3) as work,
            tc.tile_pool(name="accp", bufs=2) as accp,
            tc.tile_pool(name="outp", bufs=2) as outp,
        ):
            w_sb = consts.tile([128, 8, 128], f16)
            nc.sync.dma_start(out=w_sb, in_=w_d[:].transpose([1, 0, 2]))

            for bt in range(NBATCH):
                tiles = []
                for t in range(4):  # each tile holds chunks 2t (p 0:64), 2t+1 (p 64:128)
                    it = inp.tile([128, IPB, W], f16)
                    for half in range(2):
                        ch = 2 * t + half
                        # software-DGE DMA casts fp32 -> fp16 in flight
                        nc.gpsimd.dma_start(
                            out=it[64 * half : 64 * half + 64],
                            in_=x_d[
                                bt * IPB : (bt + 1) * IPB, 64 * ch : 64 * ch + 64, :
                            ].transpose([1, 0, 2]),
                        )
                    tiles.append(it)

                acc = accp.tile([48, NFREE], f32)
                for chunk in range(NCHUNK):
                    it = tiles[chunk // 2]
                    base = 64 * (chunk % 2)
                    # [64, img, gj, s] -> stride-8 column-phase slices
                    rhs_v = it[base : base + 64].rearrange("k i (g s) -> k i g s", s=8)
                    ps = psum_pool.tile([128, NFREE], f32)
                    for s in range(8):
                        nc.tensor.matmul(
                            ps,
                            w_sb[base : base + 64, s, :],
                            rhs_v[:, :, :, s],
                            start=(s == 0),
                            stop=(s == 7),
                        )
                    sq_re = work.tile([48, NFREE], f32)
                    sq_im = work.tile([48, NFREE], f32)
                    nc.scalar.square(sq_re, ps[0:48])
                    nc.scalar.square(sq_im, ps[64:112])
                    ss = work.tile([48, NFREE], f32)
                    nc.vector.tensor_add(ss, sq_re, sq_im)
                    if chunk == 0:
                        nc.scalar.sqrt(acc, ss)
                    else:
                        mag = work.tile([48, NFREE], f32)
                        nc.scalar.sqrt(mag, ss)
                        nc.vector.tensor_add(acc, acc, mag)

                ob = outp.tile([48, IPB], f32)
                nc.vector.reduce_sum(
                    out=ob,
                    in_=acc.rearrange("p (i g) -> p i g", g=GJ),
                    axis=mybir.AxisListType.X,
                )
                nc.sync.dma_start(
                    out=out_d[:, bt * IPB : (bt + 1) * IPB], in_=ob
                )

    nc.compile()
    return nc


def kernel(x: np.ndarray) -> np.ndarray:
    global _CACHED_NC, LAST_EXEC_NS, LAST_RESULTS
    x = np.ascontiguousarray(np.asarray(x, dtype=np.float32))
    assert x.shape == (B, C, H, W), x.shape

    if _CACHED_NC is None:
        _CACHED_NC = _build()
    nc = _CACHED_NC

    w = _weights()
    in_maps = [
        {"x": x[i * BL : (i + 1) * BL].reshape(NIMG, H, W), "w": w}
        for i in range(N_CORES)
    ]
    kwargs = dict(BENCH_KWARGS)
    if BENCH:
        kwargs.setdefault("trace", True)
    res = run_bass_kernel_spmd(nc, in_maps, core_ids=list(range(N_CORES)), **kwargs)
    LAST_EXEC_NS = res.exec_time_ns
    LAST_RESULTS = res

    outs = []
    for i in range(N_CORES):
        o = np.asarray(res.results[i]["out"], dtype=np.float64)  # [48, 24]
        o = o.reshape(NBANDS, 8, NIMG)  # [band, gi_l, img]
        o = o.sum(axis=1) / 4096.0      # mean over all 64x64 blocks
        outs.append(o.T.reshape(BL, C * NBANDS))  # img = b_l*C + ch
    return np.concatenate(outs, axis=0).astype(np.float32)
